# revision 31
# baseline (speedup 1.0000x reference)
"""MoE (64 experts, top-24) on 8 TRN2 cores — SPARSE expert dispatch.

Strategy: data-parallel over the 8192-token batch (1024 tokens/core).
Instead of computing all 64 experts densely (the old baseline), each core:
  - computes gate logits in exact fp32 + top-24 masked softmax (as before),
  - builds, per expert, the compacted list of assigned tokens on-device:
      posT = exclusive-cumsum over tokens of the selection mask (DVE
      tensor_tensor_scan on the transposed routing weights), then one
      GPSIMD local_scatter writes table[e, pos] = token_id + 1024
      (fixed capacity C=512 per expert; unused slots stay 0),
  - gathers each expert's tokens straight into the transposed [i_part, slot]
    layout with gpsimd.dma_gather(transpose=True) from a bf16 copy of x in
    DRAM (rows offset by +1024 so pad slots hit the zeroed row 0),
  - runs the expert MLP in bf16 only on C=512 slots instead of 1024 tokens,
  - multiplies h2 by the per-slot routing weight (gathered likewise from a
    wroute DRAM copy whose row 0 is zero -> pad slots contribute exactly 0),
  - scatter-adds the weighted h2 rows into a fp16 DRAM accumulator with
    gpsimd.dma_scatter_add (indices = the same token table),
  - adds the (routing-weighted) b2 bias term via one small matmul set and
    writes the final [1024, 256] fp16 output.

Top-24 of 64 means the dense baseline wastes 64/24 = 2.67x FLOPs; with
capacity padding (512 vs avg 384 tokens/expert) this kernel does ~0.5x the
dense matmul work.
"""

import sys
import types

import numpy as np

import concourse.bass as bass
import concourse.tile as tile
import concourse.mybir as mybir
from concourse import bacc, bass_utils, masks

# bass_utils imports antenv.axon_hooks when BASS_TRACE=1; some images lack it.
try:
    import antenv.axon_hooks  # noqa: F401
except ImportError:
    try:
        import contextlib
        import ctypes

        def _make_hook():
            try:
                lib = ctypes.CDLL("/opt/axon/libaxon_pjrt.so")
            except OSError:
                return None
            if not hasattr(lib, "axon_start_nrt_profile"):
                return None
            lib.axon_start_nrt_profile.argtypes = [
                ctypes.POINTER(ctypes.c_int64), ctypes.c_size_t]
            lib.axon_start_nrt_profile.restype = ctypes.c_int64
            lib.axon_stop_nrt_profile.argtypes = [ctypes.c_char_p]
            lib.axon_stop_nrt_profile.restype = ctypes.c_int64

            @contextlib.contextmanager
            def _hook(output_dir, device_ids):
                import jax
                jax.devices()
                if device_ids:
                    ids = (ctypes.c_int64 * len(device_ids))(*device_ids)
                    rc = lib.axon_start_nrt_profile(ids, len(device_ids))
                else:
                    rc = lib.axon_start_nrt_profile(None, 0)
                if rc != 0:
                    raise RuntimeError(f"axon_start_nrt_profile rc={rc}")
                try:
                    yield
                finally:
                    lib.axon_stop_nrt_profile(str(output_dir).encode())

            return _hook

        _mod = types.ModuleType("antenv.axon_hooks")
        _mod.get_axon_ntff_profile_hook = _make_hook
        _mod.set_axon_ntff_profile_hook = lambda h: None
        sys.modules["antenv.axon_hooks"] = _mod
    except Exception:
        pass

F32 = mybir.dt.float32
F16 = mybir.dt.float16
BF16 = mybir.dt.bfloat16
I16 = mybir.dt.int16
AF = mybir.ActivationFunctionType
ALU = mybir.AluOpType
AX = mybir.AxisListType

NCORES = 8
B = 8192
D = 1024          # input dim
H = 256           # hidden dim
O = 256           # output dim
NE = 64           # experts
TOPK = 24
BS = B // NCORES  # tokens per core (1024)
NBT = BS // 128   # 128-token tiles per core (8)
NG = BS // 512    # 512-token groups (gate matmul) (2)
KC = D // 128     # contraction chunks (8)
HC = H // 128     # hidden chunks (2)
C = 512           # token capacity per expert (mean 384, sd ~15.5)
EG = 4            # experts per gather/scatter group
NGRP = NE // EG   # 16 groups
CN = C * EG       # slots per group (2048)
XROWS = BS + BS   # gather-source rows: [0..1024) zeros, [1024..2048) = x

_CACHE = {}


def _build(num_devices=NCORES, debug_taps=False, stage=4, acc_dt=None):
    """stage: 1=routing+tables only, 2=+gathers, 3=+compute, 4=full."""
    ACC = F16 if acc_dt is None else acc_dt
    nc = bacc.Bacc("TRN2", target_bir_lowering=False, debug=False,
                   num_devices=num_devices)

    xt_d = nc.dram_tensor("xt", (D, BS), F32, kind="ExternalInput").ap()
    xb_d = nc.dram_tensor("xb", (XROWS, D), BF16, kind="ExternalInput").ap()
    gw_d = nc.dram_tensor("gw", (D, NE), F32, kind="ExternalInput").ap()
    gb_d = nc.dram_tensor("gb", (NE, 1), F32, kind="ExternalInput").ap()
    w1_d = nc.dram_tensor("w1", (NE, D, H), BF16, kind="ExternalInput").ap()
    b1_d = nc.dram_tensor("b1p", (128, HC * NE), F32, kind="ExternalInput").ap()
    w2_d = nc.dram_tensor("w2", (NE, H, O), BF16, kind="ExternalInput").ap()
    b2_d = nc.dram_tensor("b2", (NE, O), F32, kind="ExternalInput").ap()
    out_d = nc.dram_tensor("out", (BS, O), F16, kind="ExternalOutput").ap()
    # internal scratch
    tbl_d = nc.dram_tensor("tbl", (NE, C), I16, kind="Internal").ap()
    if debug_taps:
        dxg_d = nc.dram_tensor("dxg", (128, CN // 128, KC, 128), BF16,
                               kind="ExternalOutput").ap()
        dh2_d = nc.dram_tensor("dh2", (128, 16, O), F16,
                               kind="ExternalOutput").ap()
        ds1_d = nc.dram_tensor("ds1", (128, HC, C), BF16,
                               kind="ExternalOutput").ap()
    wr2_d = nc.dram_tensor("wr2", (XROWS, NE), F32, kind="Internal").ap()
    oac_d = nc.dram_tensor("oac", (XROWS, O), ACC, kind="Internal").ap()

    with tile.TileContext(nc) as tc:
        with tc.tile_pool(name="res", bufs=1) as res, \
             tc.tile_pool(name="rt", bufs=2) as rt, \
             tc.tile_pool(name="sc", bufs=1) as sc, \
             tc.tile_pool(name="w1p", bufs=3) as w1p, \
             tc.tile_pool(name="w2p", bufs=3) as w2p, \
             tc.tile_pool(name="tbp", bufs=2) as tbp, \
             tc.tile_pool(name="xgp", bufs=1) as xgp, \
             tc.tile_pool(name="wgp", bufs=1) as wgp, \
             tc.tile_pool(name="s1p", bufs=3) as s1p, \
             tc.tile_pool(name="h2p", bufs=1) as h2p, \
             tc.tile_pool(name="ph1p", bufs=4, space="PSUM") as ph1p, \
             tc.tile_pool(name="ph2p", bufs=2, space="PSUM") as ph2p:

            # ---------------- resident loads ----------------
            gw_sb = res.tile([128, KC, NE], F32)
            nc.sync.dma_start(gw_sb[:], gw_d.rearrange("(c p) n -> p c n", p=128))
            gb_sb = res.tile([NE, 1], F32)
            nc.sync.dma_start(gb_sb[:], gb_d[:])
            xt_f = res.tile([128, KC, BS], F32)
            for ic in range(KC):
                nc.sync.dma_start(xt_f[:, ic, :],
                                  xt_d[ic * 128:(ic + 1) * 128, :])
            b1_sb = res.tile([128, HC * NE], F32)
            nc.sync.dma_start(b1_sb[:], b1_d[:])
            b2_sb = res.tile([NE, O], F32)
            nc.sync.dma_start(b2_sb[:], b2_d[:])
            ident = res.tile([128, 128], F32)
            masks.make_identity(nc, ident[:])

            # zero-init DRAM accumulator + wroute row 0 (pad-slot target)
            zro = res.tile([128, NBT, O], ACC)
            nc.vector.memset(zro[:], 0.0)
            nc.sync.dma_start(
                oac_d[0:BS].rearrange("(b p) o -> p b o", p=128), zro[:])
            nc.sync.dma_start(
                oac_d[BS:XROWS].rearrange("(b p) o -> p b o", p=128), zro[:])
            zrow = res.tile([128, NBT, NE], F32)
            nc.vector.memset(zrow[:], 0.0)
            nc.sync.dma_start(
                wr2_d[0:BS].rearrange("(b p) e -> p b e", p=128), zrow[:])

            def load_expert(e):
                w1_t = w1p.tile([128, KC, H], BF16, tag="w1", name=f"w1_{e}")
                nc.sync.dma_start(
                    w1_t[:], w1_d[e].rearrange("(c p) h -> p c h", p=128))
                w2_t = w2p.tile([128, HC, O], BF16, tag="w2", name=f"w2_{e}")
                nc.sync.dma_start(
                    w2_t[:], w2_d[e].rearrange("(c p) o -> p c o", p=128))
                return w1_t, w2_t

            preload = {e: load_expert(e) for e in range(2)}

            g_sb = res.tile([128, NBT, NE], F32)       # gate logits [tok, e]
            wroute = res.tile([128, NBT, NE], F32)     # routing weights
            wrouteT = res.tile([64, BS], F32)          # [e, tok]
            b2term = res.tile([128, NBT, O], F16)      # sum_e w(t,e) b2[e]

            # ---------------- gate logits (exact fp32) ----------------
            gT_sb = res.tile([64, NG, 512], F32)
            for g in range(NG):
                pgt = ph1p.tile([128, 512], F32, tag="ph1", name=f"pgt_{g}")
                for ic in range(KC):
                    nc.tensor.matmul(
                        pgt[0:NE, :],
                        gw_sb[:, ic, :],
                        xt_f[:, ic, g * 512:(g + 1) * 512],
                        start=(ic == 0), stop=(ic == KC - 1))
                nc.scalar.activation(gT_sb[:, g, :], pgt[0:NE, :],
                                     AF.Identity, bias=gb_sb[:], scale=1.0)
                for btl in range(4):
                    bt = g * 4 + btl
                    ptg = ph1p.tile([128, 512], F32, tag="ph1",
                                    name=f"ptg_{bt}")
                    nc.tensor.transpose(
                        ptg[:, 0:NE],
                        gT_sb[:, g, btl * 128:(btl + 1) * 128],
                        ident[0:NE, 0:NE])
                    nc.scalar.copy(g_sb[:, bt, :], ptg[:, 0:NE])

            # ---------------- top-24 masked softmax ----------------
            for bt in range(NBT):
                g = g_sb[:, bt, :]
                m8 = rt.tile([128, 3, 8], F32, tag="m8")
                gwk = rt.tile([128, 3, NE], F32, tag="gwk")
                nc.vector.max(m8[:, 0, :], g)
                nc.vector.match_replace(gwk[:, 0, :], m8[:, 0, :], g, -1e30)
                nc.vector.max(m8[:, 1, :], gwk[:, 0, :])
                nc.vector.match_replace(gwk[:, 1, :], m8[:, 1, :], gwk[:, 0, :], -1e30)
                nc.vector.max(m8[:, 2, :], gwk[:, 1, :])
                nc.vector.match_replace(gwk[:, 2, :], m8[:, 2, :], gwk[:, 1, :], -1e30)
                maskt = rt.tile([128, NE], F32, tag="maskt")
                nc.vector.tensor_scalar(maskt[:], gwk[:, 2, :], -1e29, None,
                                        op0=ALU.is_lt)
                negm1 = rt.tile([128, 1], F32, tag="negm1")
                nc.vector.tensor_scalar_mul(negm1[:], m8[:, 0, 0:1], -1.0)
                e_sb = rt.tile([128, NE], F32, tag="e_sb")
                nc.scalar.activation(e_sb[:], g, AF.Exp, bias=negm1[:], scale=1.0)
                em = rt.tile([128, NE], F32, tag="em")
                nc.vector.tensor_mul(em[:], e_sb[:], maskt[:])
                ssum = rt.tile([128, 1], F32, tag="ssum")
                nc.vector.reduce_sum(ssum[:], em[:], axis=AX.X)
                rsum = rt.tile([128, 1], F32, tag="rsum")
                nc.vector.reciprocal(rsum[:], ssum[:])
                nc.vector.tensor_scalar_mul(wroute[:, bt, :], em[:], rsum[:])

            # wroute rows -> DRAM (slot-weight gather source), rows 1024..2047
            nc.sync.dma_start(
                wr2_d[BS:XROWS].rearrange("(b p) e -> p b e", p=128),
                wroute[:])

            # wrouteT via PE transposes
            for bt in range(NBT):
                ptr_ = ph1p.tile([128, 512], F32, tag="ph1", name=f"ptr_{bt}")
                nc.tensor.transpose(ptr_[0:64, 0:128], wroute[:, bt, :],
                                    ident[:])
                nc.scalar.copy(wrouteT[:, bt * 128:(bt + 1) * 128],
                               ptr_[0:64, 0:128])

            # b2 term: b2term[t, :] = sum_e wroute[t, e] * b2[e, :]
            for bt in range(NBT):
                pb2 = ph1p.tile([128, 512], F32, tag="ph1", name=f"pb2_{bt}")
                nc.tensor.matmul(pb2[:, 0:O],
                                 wrouteT[:, bt * 128:(bt + 1) * 128],
                                 b2_sb[:], start=True, stop=True)
                nc.scalar.copy(b2term[:, bt, :], pb2[:, 0:O])

            # ---------------- per-expert token tables ----------------
            # exclusive cumsum over tokens of the selection mask, per expert.
            # sc pool is bufs=1; tags sA/sB are reused (rotated) slots.
            maskT = sc.tile([64, BS], F32, tag="maskT")
            nc.vector.tensor_scalar(maskT[:], wrouteT[:], 0.0, None,
                                    op0=ALU.is_gt)
            zeros64 = sc.tile([64, BS], F32, tag="zeros64")
            nc.vector.memset(zeros64[:], 0.0)
            cums = sc.tile([64, BS], F32, tag="sA", name="cums")
            nc.vector.tensor_tensor_scan(cums[:], maskT[:], zeros64[:], 0.0,
                                         op0=ALU.add, op1=ALU.add)
            posx = sc.tile([64, BS], F32, tag="sB", name="posx")
            nc.vector.tensor_tensor(posx[:], cums[:], maskT[:],
                                    op=ALU.subtract)
            pos1 = sc.tile([64, BS], F32, tag="sA", name="pos1")
            nc.vector.scalar_tensor_tensor(pos1[:], posx[:], 1.0, maskT[:],
                                           op0=ALU.add, op1=ALU.mult)
            posMf = sc.tile([64, BS], F32, tag="sB", name="posMf")
            nc.vector.tensor_scalar_add(posMf[:], pos1[:], -1.0)
            posM16 = sc.tile([64, BS], I16, tag="posM16")
            nc.vector.tensor_copy(posM16[:], posMf[:])
            # token-id ramp BS..BS+1023 via scan over ones (iota is gpsimd-only)
            ones64 = sc.tile([64, BS], F32, tag="sA", name="ones64")
            nc.vector.memset(ones64[:], 1.0)
            ramp = sc.tile([64, BS], F32, tag="sB", name="ramp")
            nc.vector.tensor_tensor_scan(ramp[:], ones64[:], zeros64[:],
                                         float(BS - 1),
                                         op0=ALU.add, op1=ALU.add)
            toki = sc.tile([64, BS], I16, tag="toki")
            nc.vector.tensor_copy(toki[:], ramp[:])
            table2 = sc.tile([64, C], I16, tag="table2")
            nc.gpsimd.local_scatter(table2[:], toki[:], posM16[:],
                                    channels=64, num_elems=C, num_idxs=BS)
            nc.sync.dma_start(tbl_d[:], table2[:])

            # ---------------- sparse expert loop ----------------
            NCH = CN // 128      # 128-slot gather chunks per group (16)
            CB = C // 128        # slot blocks per expert (4)
            for grp in range(NGRP if stage >= 2 else 0):
                e0 = grp * EG
                # wrapped+replicated idx tile [128, EG, C/16] for the Ant
                # transpose-gather (hw reads idx j from partition j%16,
                # replicated for each of the 8 Q7 cores)
                tblw = tbp.tile([128, EG, C // 16], I16, tag="tblw",
                                name=f"tblw_{grp}")
                src = tbl_d[e0:e0 + EG].rearrange("k (s w) -> w k s", w=16)
                for r in range(8):
                    nc.sync.dma_start(tblw[16 * r:16 * (r + 1)], src)
                # gather x rows, transposed -> [i_part, chunk, ic, slot%128]
                # (SWDGE ring holds <=128 descriptors -> 128 idxs per op)
                xgT = xgp.tile([128, NCH, KC, 128], BF16, tag="xgT",
                               name=f"xgT_{grp}")
                for q in range(NCH):
                    nc.gpsimd.dma_gather(
                        xgT[:, q], xb_d[:],
                        tblw[:, q // CB, 8 * (q % CB):8 * (q % CB) + 8],
                        128, 128, D, transpose=True)
                # gather routing-weight rows -> [slot%128, slotblk, e]
                # (Ant non-transpose gather; 1024 idxs = 65 descs fits ring)
                wg = wgp.tile([128, EG * CB, NE], F32, tag="wg",
                              name=f"wg_{grp}")
                for h in range(2):
                    nc.gpsimd.dma_gather(
                        wg[:, 8 * h:8 * (h + 1), :], wr2_d[:],
                        tblw[:, 2 * h:2 * h + 2, :], CN // 2, CN // 2, NE,
                        transpose=False)
                h2w = h2p.tile([128, EG * CB, O], ACC, tag="h2w",
                               name=f"h2w_{grp}")
                for k in range(EG if stage >= 3 else 0):
                    e = e0 + k
                    w1_t, w2_t = preload.pop(e) if e in preload else \
                        load_expert(e)
                    if e + 2 < NE and (e + 2) not in preload:
                        preload[e + 2] = load_expert(e + 2)
                    ph1 = [ph1p.tile([128, C], F32, tag="ph1",
                                     name=f"ph1_{e}_{hc}")
                           for hc in range(HC)]
                    for ic in range(KC):
                        for hc in range(HC):
                            nc.tensor.matmul(
                                ph1[hc][:],
                                w1_t[:, ic, hc * 128:(hc + 1) * 128],
                                xgT[:, 4 * k:4 * k + 4, ic, :],
                                start=(ic == 0), stop=(ic == KC - 1))
                    s1 = s1p.tile([128, HC, C], BF16, tag="s1",
                                  name=f"s1_{e}")
                    for hc in range(HC):
                        nc.scalar.activation(
                            s1[:, hc, :], ph1[hc][:], AF.Relu,
                            bias=b1_sb[:, hc * NE + e: hc * NE + e + 1],
                            scale=1.0)
                    if debug_taps and e == 0:
                        nc.sync.dma_start(ds1_d[:], s1[:])
                    ph2 = ph2p.tile([128, CB, O], F32, tag="ph2",
                                    name=f"ph2_{e}")
                    for cb in range(CB):
                        for hc in range(HC):
                            nc.tensor.matmul(
                                ph2[:, cb, :],
                                s1[:, hc, cb * 128:(cb + 1) * 128],
                                w2_t[:, hc, :],
                                start=(hc == 0), stop=(hc == HC - 1))
                    for cb in range(CB):
                        j = k * CB + cb
                        nc.vector.tensor_scalar_mul(
                            h2w[:, j, :], ph2[:, cb, :], wg[:, j, e:e + 1])
                    # scatter-add this expert's weighted outputs into the
                    # token accumulator. Per-expert (not per-group): one
                    # expert's slots are distinct tokens, so no duplicate
                    # indices within an op (duplicates would collide in the
                    # RMW); across ops the WAW dependency serializes.
                    if stage >= 4:
                        # production Ant scatter-add ucode (per-expert: 512
                        # idxs -> m2s=65 fits the 128-entry SWDGE ring; no
                        # duplicate rows within one expert)
                        nc.gpsimd.dma_scatter_add(
                            oac_d[:], h2w[:, k * CB:(k + 1) * CB, :],
                            tblw[:, k, :], C, C, O)
                if debug_taps and grp == 0:
                    nc.sync.dma_start(dxg_d[:], xgT[:])
                    nc.sync.dma_start(dh2_d[:], h2w[:])

            # ---------------- final: accumulator + b2 term ----------------
            if stage >= 4:
                oacc = res.tile([128, NBT, O], ACC)
                nc.sync.dma_start(
                    oacc[:],
                    oac_d[BS:XROWS].rearrange("(b p) o -> p b o", p=128))
                outs = res.tile([128, NBT, O], F16)
                nc.vector.tensor_tensor(outs[:], oacc[:], b2term[:],
                                        op=ALU.add)
            else:
                outs = b2term
            nc.sync.dma_start(
                out_d.rearrange("(b p) o -> p b o", p=128), outs[:])

    nc.compile()
    return nc


def _prep_host(gate_b, expert_b1):
    gb = np.ascontiguousarray(np.asarray(gate_b, dtype=np.float32).reshape(NE, 1))
    b1 = np.asarray(expert_b1, dtype=np.float32)          # [64, 256]
    b1p = np.ascontiguousarray(
        b1.reshape(NE, HC, 128).transpose(2, 1, 0).reshape(128, HC * NE))
    return gb, b1p


def kernel(x, gate_w, gate_b, expert_w1, expert_b1, expert_w2, expert_b2, k):
    import ml_dtypes
    assert int(k) == TOPK
    if "nc" not in _CACHE:
        _CACHE["nc"] = _build()
    nc = _CACHE["nc"]

    x = np.asarray(x, dtype=np.float32)
    gw = np.ascontiguousarray(np.asarray(gate_w, dtype=np.float32))
    w1 = np.ascontiguousarray(
        np.asarray(expert_w1, dtype=np.float32).astype(ml_dtypes.bfloat16))
    w2 = np.ascontiguousarray(
        np.asarray(expert_w2, dtype=np.float32).astype(ml_dtypes.bfloat16))
    b2 = np.ascontiguousarray(np.asarray(expert_b2, dtype=np.float32))
    gb, b1p = _prep_host(gate_b, expert_b1)

    in_maps = []
    for c in range(NCORES):
        xs = x[c * BS:(c + 1) * BS]
        xt = np.ascontiguousarray(xs.T)
        xb = np.zeros((XROWS, D), dtype=ml_dtypes.bfloat16)
        xb[BS:] = xs.astype(ml_dtypes.bfloat16)
        in_maps.append({"xt": xt, "xb": xb, "gw": gw, "gb": gb, "w1": w1,
                        "b1p": b1p, "w2": w2, "b2": b2})

    r = bass_utils.run_bass_kernel_spmd(nc, in_maps, core_ids=list(range(NCORES)))
    _CACHE["last_result"] = r
    return np.concatenate(
        [m["out"].astype(np.float32) for m in r.results], axis=0)
